# revision 7
# baseline (speedup 1.0000x reference)
"""Fused transformer block (LN -> causal MHA -> residual -> LN -> SiLU MLP -> residual)
on 8 Trainium2 NeuronCores.

Sharding: tensor-parallel over heads (2 heads/core) for QKV/attention/O-projection,
one ReduceScatter of the partial O-projection over tokens, then token-parallel MLP
(512 tokens/core, full width, weights replicated). LayerNorm affine params are folded
into the adjacent projection weights on the host, so the device only computes
(x - mean) * rsqrt(var + eps).

All matmuls run in bf16 with f32 PSUM accumulation. Residual path stays f32.
"""

import sys
import os

for _p in ("/opt/trn_rl_repo", "/root/.axon_site/_ro/trn_rl_repo"):
    if os.path.isdir(_p) and _p not in sys.path:
        sys.path.insert(0, _p)
        break

import numpy as np
import ml_dtypes

import concourse.bass as bass
from concourse import bacc
import concourse.mybir as mybir
import concourse.tile as tile
from concourse.masks import make_identity
from concourse.bass_utils import run_bass_kernel_spmd

F32 = mybir.dt.float32
BF16 = mybir.dt.bfloat16

P = 128          # partitions / head_dim / token tile
H = 2048         # hidden
KS = H // P      # 16 k-subtiles over hidden
HEADS = 16
HL = 2           # heads per core
NCORES = 8
B = 2
T = 2048
NTOK = B * T     # 4096
TPB = T          # tokens per batch
MID = 4 * H      # 8192
MMT = MID // P   # 64 m-tiles over mid dim
DQK = 2 * HL * P   # 512 rows of fused QK projection per core
DV = HL * P        # 256 V/attention-out features per core
EPS = 1e-5
NEG = -1.0e30

QT_PER_B = TPB // P   # 16 q tiles per batch
MT = NTOK // P        # 32 token m-tiles
NCHUNK = 4            # reduce-scatter chunks (1024 tokens each)
TOKC = NTOK // NCHUNK // NCORES  # 128 tokens per core per chunk


def build(sim=False, trn_kwargs=None):
    nc = bacc.Bacc(None, num_devices=NCORES, **(trn_kwargs or {}))

    x_d = nc.declare_dram_parameter("x", [NTOK, H], F32, isOutput=False)
    xres_d = nc.declare_dram_parameter("xres", [NCHUNK * TOKC, H], F32, isOutput=False)
    wqk_d = nc.declare_dram_parameter("wqk", [P, KS, DQK], BF16, isOutput=False)
    bqk_d = nc.declare_dram_parameter("bqk", [P, DQK // P], F32, isOutput=False)
    wv_d = nc.declare_dram_parameter("wv", [P, KS, DV], BF16, isOutput=False)
    bvbc_d = nc.declare_dram_parameter("bvbc", [P, DV], F32, isOutput=False)
    wo_d = nc.declare_dram_parameter("wo", [P, DV // P, H], BF16, isOutput=False)
    w1_d = nc.declare_dram_parameter("w1", [MMT, P, KS, P], BF16, isOutput=False)
    b1_d = nc.declare_dram_parameter("b1", [P, MMT], F32, isOutput=False)
    w2_d = nc.declare_dram_parameter("w2", [MID, H], BF16, isOutput=False)
    b2bc_d = nc.declare_dram_parameter("b2bc", [P, H], F32, isOutput=False)
    cmask_d = nc.declare_dram_parameter("cmask", [P, P], F32, isOutput=False)
    out_d = nc.declare_dram_parameter("out", [NCHUNK * TOKC, H], F32, isOutput=True)

    from contextlib import ExitStack
    with tile.TileContext(nc) as tc:
        with ExitStack() as stack:
            dram = stack.enter_context(tc.tile_pool(name="dram", bufs=1, space="DRAM"))
            const = stack.enter_context(tc.tile_pool(name="const", bufs=1))
            wbig = stack.enter_context(tc.tile_pool(name="wbig", bufs=1))
            p_x = stack.enter_context(tc.tile_pool(name="xin", bufs=2))
            p_ln = stack.enter_context(tc.tile_pool(name="lnsmall", bufs=3))
            p_h = stack.enter_context(tc.tile_pool(name="htok", bufs=2))
            p_hT = stack.enter_context(tc.tile_pool(name="hT", bufs=2))
            p_k = stack.enter_context(tc.tile_pool(name="ksb", bufs=2))
            p_qv = stack.enter_context(tc.tile_pool(name="qvsl", bufs=3))
            p_at = stack.enter_context(tc.tile_pool(name="attn", bufs=2))
            p_S = stack.enter_context(tc.tile_pool(name="srow", bufs=1))
            p_blk = stack.enter_context(tc.tile_pool(name="blk", bufs=3))
            p_mlp = stack.enter_context(tc.tile_pool(name="mlp", bufs=2))
            p_st = stack.enter_context(tc.tile_pool(name="stream", bufs=2))
            psA = stack.enter_context(tc.tile_pool(name="psA", bufs=4, space="PSUM"))
            psB = stack.enter_context(tc.tile_pool(name="psB", bufs=4, space="PSUM"))

            # ---- internal DRAM ----
            qT_dram = dram.tile([DV, NTOK], BF16)
            v_dram = dram.tile([NTOK, DV], BF16)
            po_dram = dram.tile([NTOK, H], BF16)
            rs_dram = dram.tile([NCHUNK * TOKC, H], BF16)
            ut_dram = dram.tile([MID, NCHUNK * TOKC], BF16)
            x2_dram = dram.tile([NCHUNK * TOKC, H], F32)

            # ---- constants / weights in SBUF ----
            ident = const.tile([P, P], BF16)
            make_identity(nc, ident)
            epsb = const.tile([P, 1], F32)
            nc.vector.memset(epsb[:], EPS)
            cmask = const.tile([P, P], F32)
            nc.sync.dma_start(cmask[:], cmask_d[:, :])
            bqk_sb = const.tile([P, DQK // P], F32)
            nc.sync.dma_start(bqk_sb[:], bqk_d[:, :])
            bvbc_sb = const.tile([P, DV], F32)
            nc.sync.dma_start(bvbc_sb[:], bvbc_d[:, :])
            b1_sb = const.tile([P, MMT], F32)
            nc.sync.dma_start(b1_sb[:], b1_d[:, :])
            b2bc_sb = const.tile([P, H], F32)
            nc.sync.dma_start(b2bc_sb[:], b2bc_d[:, :])
            wqk_sb = wbig.tile([P, KS, DQK], BF16)
            nc.sync.dma_start(wqk_sb[:], wqk_d[:, :, :])
            wv_sb = wbig.tile([P, KS, DV], BF16)
            nc.sync.dma_start(wv_sb[:], wv_d[:, :, :])
            wo_sb = wbig.tile([P, DV // P, H], BF16)
            nc.sync.dma_start(wo_sb[:], wo_d[:, :, :])

            def layer_norm_tile(xt, name):
                """xt: [P, H] f32 SBUF -> returns bf16 [P, H] normalized tile."""
                st = p_ln.tile([P, 4, 6], F32, tag="lnst", name=f"st_{name}")
                for a in range(4):
                    nc.vector.bn_stats(st[:, a, :], xt[:, 512 * a:512 * (a + 1)])
                mv = p_ln.tile([P, 2], F32, tag="lnmv", name=f"mv_{name}")
                nc.vector.bn_aggr(mv[:], st[:])
                sd = p_ln.tile([P, 1], F32, tag="lnsd", name=f"sd_{name}")
                nc.scalar.activation(sd[:], mv[:, 1:2],
                                     mybir.ActivationFunctionType.Sqrt, bias=epsb[:])
                rstd = p_ln.tile([P, 1], F32, tag="lnrstd", name=f"rstd_{name}")
                nc.vector.reciprocal(rstd[:], sd[:])
                nmu = p_ln.tile([P, 1], F32, tag="lnnmu", name=f"nmu_{name}")
                nc.vector.tensor_tensor(nmu[:], mv[:, 0:1], rstd[:], mybir.AluOpType.mult)
                nc.vector.tensor_scalar_mul(nmu[:], nmu[:], -1.0)
                ht = p_h.tile([P, H], BF16, tag="ht", name=f"ht_{name}")
                nc.scalar.activation(ht[:], xt[:],
                                     mybir.ActivationFunctionType.Identity,
                                     bias=nmu[:], scale=rstd[:])
                return ht

            # ================= Phase 1+2: LN1, transpose, QKV =================
            ksb = [None, None]   # per-batch K tiles [P, 2, TPB] bf16
            hts = [None] * (NTOK // 512)

            for nt in range(NTOK // 512):   # 512-token groups
                b = nt // 4
                if nt % 4 == 0:
                    ksb[b] = p_k.tile([P, HL, TPB], BF16, tag="ksb",
                                      name=f"ksb_{b}")
                hT = p_hT.tile([P, KS, 512], BF16, tag="hT", name=f"hT_{nt}")
                hts[nt] = hT
                for tt in range(4):         # 128-token LN tiles
                    t = 4 * nt + tt
                    xt = p_x.tile([P, H], F32, tag="xt", name=f"xt_{t}")
                    nc.sync.dma_start(xt[:], x_d[P * t:P * (t + 1), :])
                    ht = layer_norm_tile(xt, f"ln1_{t}")
                    for f in range(KS):
                        ptp = psB.tile([P, P], BF16, tag="psB", name=f"trp_{t}_{f}")
                        nc.tensor.transpose(ptp[:], ht[:, P * f:P * (f + 1)], ident[:])
                        nc.any.tensor_copy(out=hT[:, f, P * tt:P * (tt + 1)], in_=ptp[:])

                # QK projection: out rows m (0,1 -> Q head0/1 ; 2,3 -> K head0/1)
                col0 = 512 * (nt % 4)
                for m in range(4):
                    ps = psA.tile([P, 512], F32, tag="psA", name=f"qk_{nt}_{m}")
                    for ks in range(KS):
                        nc.tensor.matmul(ps[:], lhsT=wqk_sb[:, ks, P * m:P * (m + 1)],
                                         rhs=hT[:, ks, :],
                                         start=(ks == 0), stop=(ks == KS - 1))
                    if m < 2:
                        qsl = p_qv.tile([P, 512], BF16, tag="qsl", name=f"qsl_{nt}_{m}")
                        nc.scalar.activation(qsl[:], ps[:],
                                             mybir.ActivationFunctionType.Identity,
                                             bias=bqk_sb[:, m:m + 1])
                        nc.sync.dma_start(qT_dram[P * m:P * (m + 1),
                                                  512 * nt:512 * (nt + 1)], qsl[:])
                    else:
                        nc.scalar.activation(ksb[b][:, m - 2, col0:col0 + 512], ps[:],
                                             mybir.ActivationFunctionType.Identity,
                                             bias=bqk_sb[:, m:m + 1])
                # V projection (token-major)
                for m in range(4):
                    ps = psA.tile([P, 512], F32, tag="psA", name=f"v_{nt}_{m}")
                    for ks in range(KS):
                        nc.tensor.matmul(ps[:, :DV], lhsT=hT[:, ks, P * m:P * (m + 1)],
                                         rhs=wv_sb[:, ks, :],
                                         start=(ks == 0), stop=(ks == KS - 1))
                    vsl = p_qv.tile([P, DV], BF16, tag="vsl", name=f"vsl_{nt}_{m}")
                    nc.vector.tensor_tensor(vsl[:], ps[:, :DV], bvbc_sb[:],
                                            mybir.AluOpType.add)
                    r0 = 512 * nt + P * m
                    nc.sync.dma_start(v_dram[r0:r0 + P, :], vsl[:])

            # ================= Phase 3: attention + O-projection ==============
            rg = [list(range(NCORES))]
            for b in range(B):
                for qt in range(QT_PER_B):
                    mt = QT_PER_B * b + qt          # token m-tile index
                    klen = P * (qt + 1)
                    tok0 = TPB * b
                    aot = p_at.tile([P, HL, P], BF16, tag="aot", name=f"aot_{mt}")
                    for lh in range(HL):
                        qblk = p_blk.tile([P, P], BF16, tag="qblk",
                                          name=f"qb_{mt}_{lh}")
                        nc.sync.dma_start(
                            qblk[:], qT_dram[P * lh:P * (lh + 1),
                                             tok0 + P * qt:tok0 + P * (qt + 1)])
                        S = p_S.tile([P, TPB], F32, tag="S", name=f"S_{mt}_{lh}")
                        nchs = (qt + 4) // 4
                        for j in range(nchs):
                            n0 = 512 * j
                            n1 = min(n0 + 512, klen)
                            ps = psA.tile([P, 512], F32, tag="psA",
                                          name=f"s_{mt}_{lh}_{j}")
                            nc.tensor.matmul(ps[:, :n1 - n0], lhsT=qblk[:],
                                             rhs=ksb[b][:, lh, n0:n1],
                                             start=True, stop=True)
                            nc.any.tensor_copy(out=S[:, n0:n1], in_=ps[:, :n1 - n0])
                        # causal mask on the diagonal block
                        nc.vector.tensor_tensor(S[:, klen - P:klen], S[:, klen - P:klen],
                                                cmask[:], mybir.AluOpType.add)
                        negmax = p_ln.tile([P, 1], F32, tag="negmax",
                                           name=f"nm_{mt}_{lh}")
                        nc.vector.tensor_reduce(negmax[:], S[:, :klen],
                                                axis=mybir.AxisListType.X,
                                                op=mybir.AluOpType.max, negate=True)
                        pex = p_at.tile([P, TPB], BF16, tag="pex",
                                        name=f"pex_{mt}_{lh}")
                        sexp = p_ln.tile([P, 1], F32, tag="sexp",
                                         name=f"se_{mt}_{lh}")
                        nc.scalar.activation(pex[:, :klen], S[:, :klen],
                                             mybir.ActivationFunctionType.Exp,
                                             bias=negmax[:], accum_out=sexp[:])
                        rinv = p_ln.tile([P, 1], F32, tag="rinv",
                                         name=f"ri_{mt}_{lh}")
                        nc.vector.reciprocal(rinv[:], sexp[:])
                        nc.vector.tensor_scalar_mul(pex[:, :klen], pex[:, :klen],
                                                    rinv[:])
                        ps_o = psB.tile([P, P], F32, tag="psB", name=f"o_{mt}_{lh}")
                        for kb in range(qt + 1):
                            ptp = psB.tile([P, P], BF16, tag="psB",
                                           name=f"pt_{mt}_{lh}_{kb}")
                            nc.tensor.transpose(ptp[:], pex[:, P * kb:P * (kb + 1)],
                                                ident[:])
                            pts = p_blk.tile([P, P], BF16, tag="pts",
                                             name=f"pts_{mt}_{lh}_{kb}")
                            nc.any.tensor_copy(out=pts[:], in_=ptp[:])
                            vblk = p_blk.tile([P, P], BF16, tag="vblk",
                                              name=f"vb_{mt}_{lh}_{kb}")
                            nc.sync.dma_start(
                                vblk[:], v_dram[tok0 + P * kb:tok0 + P * (kb + 1),
                                                P * lh:P * (lh + 1)])
                            nc.tensor.matmul(ps_o[:], lhsT=vblk[:], rhs=pts[:],
                                             start=(kb == 0), stop=(kb == qt))
                        nc.any.tensor_copy(out=aot[:, lh, :], in_=ps_o[:])
                    # O projection partial for this 128-token tile
                    r0 = P * mt
                    for nk in range(4):
                        ps = psA.tile([P, 512], F32, tag="psA", name=f"po_{mt}_{nk}")
                        for ks in range(HL):
                            nc.tensor.matmul(ps[:], lhsT=aot[:, ks, :],
                                             rhs=wo_sb[:, ks, 512 * nk:512 * (nk + 1)],
                                             start=(ks == 0), stop=(ks == HL - 1))
                        posl = p_qv.tile([P, 512], BF16, tag="posl",
                                         name=f"posl_{mt}_{nk}")
                        nc.any.tensor_copy(out=posl[:], in_=ps[:])
                        nc.sync.dma_start(po_dram[r0:r0 + P, 512 * nk:512 * (nk + 1)],
                                          posl[:])
                    # issue reduce-scatter when a 1024-token chunk is complete
                    if mt % (MT // NCHUNK) == MT // NCHUNK - 1:
                        j = mt // (MT // NCHUNK)
                        nc.gpsimd.collective_compute(
                            "ReduceScatter", mybir.AluOpType.add, replica_groups=rg,
                            ins=[po_dram[1024 * j:1024 * (j + 1), :]],
                            outs=[rs_dram[TOKC * j:TOKC * (j + 1), :]])

            # ================= Phase 4: residual + LN2 + MLP ==================
            h2T = [None, None]   # [P, KS, 256] per chunk pair
            for j in range(NCHUNK):
                x2 = p_mlp.tile([P, H], F32, tag="x2", name=f"x2_{j}")
                nc.sync.dma_start(x2[:], xres_d[P * j:P * (j + 1), :])
                rsj = p_mlp.tile([P, H], BF16, tag="rsj", name=f"rsj_{j}")
                nc.sync.dma_start(rsj[:], rs_dram[P * j:P * (j + 1), :])
                nc.vector.tensor_tensor(x2[:], x2[:], rsj[:], mybir.AluOpType.add)
                nc.sync.dma_start(x2_dram[P * j:P * (j + 1), :], x2[:])
                h2 = layer_norm_tile(x2, f"ln2_{j}")
                jg, jj = j // 2, j % 2
                if jj == 0:
                    h2T[jg] = p_mlp.tile([P, KS, 256], BF16, tag="h2T",
                                         name=f"h2T_{jg}")
                for f in range(KS):
                    ptp = psB.tile([P, P], BF16, tag="psB", name=f"h2t_{j}_{f}")
                    nc.tensor.transpose(ptp[:], h2[:, P * f:P * (f + 1)], ident[:])
                    nc.any.tensor_copy(out=h2T[jg][:, f, P * jj:P * (jj + 1)],
                                       in_=ptp[:])

            # MLP1: U_T[mid, tok] = silu(W1_eff.T @ h2T + b1), spilled to DRAM
            silu_fn = (mybir.ActivationFunctionType.Sigmoid if sim
                       else mybir.ActivationFunctionType.Silu)
            for mm in range(MMT):
                w1t = p_st.tile([P, KS, P], BF16, tag="w1t", name=f"w1t_{mm}")
                nc.sync.dma_start(w1t[:], w1_d[mm, :, :, :])
                for jg in range(2):
                    ps = psA.tile([P, 512], F32, tag="psA", name=f"u_{mm}_{jg}")
                    for ks in range(KS):
                        nc.tensor.matmul(ps[:, :256], lhsT=w1t[:, ks, :],
                                         rhs=h2T[jg][:, ks, :],
                                         start=(ks == 0), stop=(ks == KS - 1))
                    usl = p_st.tile([P, 256], BF16, tag="usl", name=f"usl_{mm}_{jg}")
                    nc.scalar.activation(usl[:], ps[:, :256], silu_fn,
                                         bias=b1_sb[:, mm:mm + 1])
                    nc.sync.dma_start(
                        ut_dram[P * mm:P * (mm + 1), 256 * jg:256 * (jg + 1)], usl[:])

            # MLP2: out[tok, H] = U_T.T @ W2 + b2 + x2
            for jg in range(2):
                for ng in range(2):
                    pss = [[None] * 2 for _ in range(2)]
                    for jj in range(2):
                        for nn in range(2):
                            pss[jj][nn] = psA.tile([P, 512], F32, tag="psA",
                                                   name=f"y_{jg}_{ng}_{jj}_{nn}")
                    for ks in range(MMT):
                        utk = p_st.tile([P, 256], BF16, tag="utk",
                                        name=f"utk_{jg}_{ng}_{ks}")
                        nc.sync.dma_start(
                            utk[:], ut_dram[P * ks:P * (ks + 1),
                                            256 * jg:256 * (jg + 1)])
                        w2t = p_st.tile([P, 1024], BF16, tag="w2t",
                                        name=f"w2t_{jg}_{ng}_{ks}")
                        nc.sync.dma_start(
                            w2t[:], w2_d[P * ks:P * (ks + 1),
                                         1024 * ng:1024 * (ng + 1)])
                        for jj in range(2):
                            for nn in range(2):
                                nc.tensor.matmul(
                                    pss[jj][nn][:],
                                    lhsT=utk[:, P * jj:P * (jj + 1)],
                                    rhs=w2t[:, 512 * nn:512 * (nn + 1)],
                                    start=(ks == 0), stop=(ks == MMT - 1))
                    for jj in range(2):
                        j = 2 * jg + jj
                        for nn in range(2):
                            c0 = 1024 * ng + 512 * nn
                            x2sl = p_st.tile([P, 512], F32, tag="x2sl",
                                             name=f"x2sl_{j}_{ng}_{nn}")
                            nc.sync.dma_start(x2sl[:],
                                              x2_dram[P * j:P * (j + 1), c0:c0 + 512])
                            ot = p_st.tile([P, 512], F32, tag="ot",
                                           name=f"ot_{j}_{ng}_{nn}")
                            nc.vector.tensor_tensor(ot[:], pss[jj][nn][:],
                                                    b2bc_sb[:, c0:c0 + 512],
                                                    mybir.AluOpType.add)
                            nc.vector.tensor_tensor(ot[:], ot[:], x2sl[:],
                                                    mybir.AluOpType.add)
                            nc.sync.dma_start(out_d[P * j:P * (j + 1), c0:c0 + 512],
                                              ot[:])
    nc.compile()
    return nc


def _bf16(a):
    return np.asarray(a, dtype=np.float32).astype(ml_dtypes.bfloat16)


def make_in_maps(x, Wq, Wk, Wv, Wo, g1, bn1, g2, bn2, W1, b1, W2, b2):
    x = np.asarray(x, np.float32)
    x_flat = np.ascontiguousarray(x.reshape(NTOK, H))
    s = np.float32(1.0 / np.sqrt(P))

    wq_eff = (g1[:, None] * np.asarray(Wq, np.float32)) * s
    wk_eff = g1[:, None] * np.asarray(Wk, np.float32)
    wv_eff = g1[:, None] * np.asarray(Wv, np.float32)
    bq = (bn1 @ np.asarray(Wq, np.float32)) * s
    bk = bn1 @ np.asarray(Wk, np.float32)
    bv = bn1 @ np.asarray(Wv, np.float32)
    w1_eff = g2[:, None] * np.asarray(W1, np.float32)
    b1_eff = np.asarray(b1, np.float32) + bn2 @ np.asarray(W1, np.float32)

    # shared tensors
    w1_t = np.ascontiguousarray(
        _bf16(w1_eff).reshape(KS, P, MMT, P).transpose(2, 1, 0, 3))  # [mm, p, ks, mw]
    w2_t = np.ascontiguousarray(_bf16(W2))
    b1m = np.ascontiguousarray(b1_eff.reshape(MMT, P).T.astype(np.float32))
    b2bc = np.ascontiguousarray(
        np.broadcast_to(np.asarray(b2, np.float32), (P, H)))
    ii, jj_ = np.meshgrid(np.arange(P), np.arange(P), indexing="ij")
    cmask = np.where(jj_ <= ii, 0.0, NEG).astype(np.float32)

    in_maps = []
    for c in range(NCORES):
        cs = slice(DV * c, DV * (c + 1))
        wqk = np.concatenate([wq_eff[:, cs], wk_eff[:, cs]], axis=1)  # [H, 512]
        wqk_t = np.ascontiguousarray(
            _bf16(wqk).reshape(KS, P, DQK).transpose(1, 0, 2))
        bqk = np.concatenate([bq[cs], bk[cs]]).astype(np.float32)
        bqk_m = np.ascontiguousarray(bqk.reshape(DQK // P, P).T)
        wv_t = np.ascontiguousarray(
            _bf16(wv_eff[:, cs]).reshape(KS, P, DV).transpose(1, 0, 2))
        bvbc = np.ascontiguousarray(
            np.broadcast_to(bv[cs].astype(np.float32), (P, DV)))
        wo_t = np.ascontiguousarray(
            _bf16(np.asarray(Wo, np.float32)[cs, :]).reshape(DV // P, P, H)
            .transpose(1, 0, 2))
        xres = np.concatenate(
            [x_flat[1024 * j + P * c:1024 * j + P * (c + 1)] for j in range(NCHUNK)],
            axis=0)
        in_maps.append({
            "x": x_flat, "xres": np.ascontiguousarray(xres),
            "wqk": wqk_t, "bqk": bqk_m, "wv": wv_t, "bvbc": bvbc, "wo": wo_t,
            "w1": w1_t, "b1": b1m, "w2": w2_t, "b2bc": b2bc, "cmask": cmask,
        })
    return in_maps


_NC_CACHE = {}


def kernel(**inputs):
    if "nc" not in _NC_CACHE:
        _NC_CACHE["nc"] = build()
    nc = _NC_CACHE["nc"]
    in_maps = make_in_maps(
        inputs["x"], inputs["Wq"], inputs["Wk"], inputs["Wv"], inputs["Wo"],
        np.asarray(inputs["g1"], np.float32), np.asarray(inputs["bn1"], np.float32),
        np.asarray(inputs["g2"], np.float32), np.asarray(inputs["bn2"], np.float32),
        inputs["W1"], inputs["b1"], inputs["W2"], inputs["b2"])
    res = run_bass_kernel_spmd(nc, in_maps, list(range(NCORES)))
    out = np.empty((NTOK, H), np.float32)
    for c in range(NCORES):
        oc = res.results[c]["out"]
        for j in range(NCHUNK):
            out[1024 * j + P * c:1024 * j + P * (c + 1)] = oc[P * j:P * (j + 1)]
    return out.reshape(B, T, H)


# revision 20
# speedup vs baseline: 3.0482x; 3.0482x over previous
"""Fused transformer block (LN -> causal MHA -> residual -> LN -> SiLU MLP -> residual)
on 8 Trainium2 NeuronCores.

Sharding: tensor-parallel over heads (2 heads/core) for QKV/attention/O-projection,
one ReduceScatter of the partial O-projection over tokens, then token-parallel MLP
(512 tokens/core, full width, weights replicated). LayerNorm affine params are folded
into the adjacent projection weights on the host, so the device only computes
(x - mean) * rsqrt(var + eps).

All matmuls run in bf16 with f32 PSUM accumulation. Residual path stays f32.
"""

import sys
import os

for _p in ("/opt/trn_rl_repo", "/root/.axon_site/_ro/trn_rl_repo"):
    if os.path.isdir(_p) and _p not in sys.path:
        sys.path.insert(0, _p)
        break

import numpy as np
import ml_dtypes

import concourse.bass as bass
from concourse import bacc
import concourse.mybir as mybir
import concourse.tile as tile
from concourse.masks import make_identity
from concourse.bass_utils import run_bass_kernel_spmd

F32 = mybir.dt.float32
BF16 = mybir.dt.bfloat16

P = 128          # partitions / head_dim / token tile
H = 2048         # hidden
KS = H // P      # 16 k-subtiles over hidden
HEADS = 16
HL = 2           # heads per core
NCORES = 8
B = 2
T = 2048
NTOK = B * T     # 4096
TPB = T          # tokens per batch
MID = 4 * H      # 8192
MMT = MID // P   # 64 m-tiles over mid dim
DQK = 2 * HL * P   # 512 rows of fused QK projection per core
DV = HL * P        # 256 V/attention-out features per core
EPS = 1e-5
NEG = -1.0e30

QT_PER_B = TPB // P   # 16 q tiles per batch
MT = NTOK // P        # 32 token m-tiles
NCHUNK = 4            # reduce-scatter chunks (1024 tokens each)
TOKC = NTOK // NCHUNK // NCORES  # 128 tokens per core per chunk


def build(sim=False, trn_kwargs=None, trace_sim=False):
    nc = bacc.Bacc(None, num_devices=NCORES, **(trn_kwargs or {}))

    x_d = nc.declare_dram_parameter("x", [NTOK, H], F32, isOutput=False)
    xres_d = nc.declare_dram_parameter("xres", [NCHUNK * TOKC, H], F32, isOutput=False)
    wqk_d = nc.declare_dram_parameter("wqk", [P, KS, DQK], BF16, isOutput=False)
    bqk_d = nc.declare_dram_parameter("bqk", [P, DQK // P], F32, isOutput=False)
    wv_d = nc.declare_dram_parameter("wv", [P, KS, DV], BF16, isOutput=False)
    bvbc_d = nc.declare_dram_parameter("bvbc", [P, DV], F32, isOutput=False)
    wo_d = nc.declare_dram_parameter("wo", [P, DV // P, H], BF16, isOutput=False)
    w1_d = nc.declare_dram_parameter("w1", [MMT, P, KS, P], BF16, isOutput=False)
    b1_d = nc.declare_dram_parameter("b1", [P, MMT], F32, isOutput=False)
    w2_d = nc.declare_dram_parameter("w2", [MID, H], BF16, isOutput=False)
    b2bc_d = nc.declare_dram_parameter("b2bc", [P, H], F32, isOutput=False)
    cmask_d = nc.declare_dram_parameter("cmask", [P, P], F32, isOutput=False)
    out_d = nc.declare_dram_parameter("out", [NCHUNK * TOKC, H], F32, isOutput=True)

    from contextlib import ExitStack
    with tile.TileContext(nc, trace_sim=trace_sim) as tc:
        with ExitStack() as stack:
            dram = stack.enter_context(tc.tile_pool(name="dram", bufs=1, space="DRAM"))
            const = stack.enter_context(tc.tile_pool(name="const", bufs=1))
            wbig = stack.enter_context(tc.tile_pool(name="wbig", bufs=1))
            p_x = stack.enter_context(tc.tile_pool(name="xin", bufs=2))
            p_ln = stack.enter_context(tc.tile_pool(name="lnsmall", bufs=3))
            p_h = stack.enter_context(tc.tile_pool(name="htok", bufs=2))
            p_hT = stack.enter_context(tc.tile_pool(name="hT", bufs=2))
            p_k = stack.enter_context(tc.tile_pool(name="ksb", bufs=2))
            p_qv = stack.enter_context(tc.tile_pool(name="qvsl", bufs=2))
            p_at = stack.enter_context(tc.tile_pool(name="attn", bufs=2))
            p_blk = stack.enter_context(tc.tile_pool(name="blk", bufs=2))
            p_mlp = stack.enter_context(tc.tile_pool(name="mlp", bufs=2))
            p_rs = stack.enter_context(tc.tile_pool(name="rspool", bufs=1))
            p_st = stack.enter_context(tc.tile_pool(name="stream", bufs=2))
            psA = stack.enter_context(tc.tile_pool(name="psA", bufs=8, space="PSUM"))

            # ---- internal DRAM ----
            qT_dram = dram.tile([DV, NTOK], BF16)
            po_dram = dram.tile([NTOK, H], BF16)
            rs_dram = dram.tile([NCHUNK * TOKC, H], BF16)
            ut_dram = dram.tile([MID, NCHUNK * TOKC], BF16)
            x2_dram = dram.tile([NCHUNK * TOKC, H], F32)

            # ---- constants / weights in SBUF ----
            ident = const.tile([P, P], BF16)
            make_identity(nc, ident)
            epsb = const.tile([P, 1], F32)
            nc.vector.memset(epsb[:], EPS)
            cmask = const.tile([P, P], F32)
            nc.sync.dma_start(cmask[:], cmask_d[:, :])
            bqk_sb = const.tile([P, DQK // P], F32)
            nc.sync.dma_start(bqk_sb[:], bqk_d[:, :])
            bvbc_sb = const.tile([P, DV], F32)
            nc.sync.dma_start(bvbc_sb[:], bvbc_d[:, :])
            b1_sb = const.tile([P, MMT], F32)
            nc.sync.dma_start(b1_sb[:], b1_d[:, :])
            wqk_sb = wbig.tile([P, KS, DQK], BF16)
            nc.sync.dma_start(wqk_sb[:], wqk_d[:, :, :])
            wv_sb = wbig.tile([P, KS, DV], BF16)
            nc.sync.dma_start(wv_sb[:], wv_d[:, :, :])
            wo_sb = wbig.tile([P, DV // P, H], BF16)
            nc.sync.dma_start(wo_sb[:], wo_d[:, :, :])

            def layer_norm_tile(xt, name):
                """xt: [P, H] f32 SBUF -> returns bf16 [P, H] normalized tile."""
                st = p_ln.tile([P, 4, 6], F32, tag="lnst", name=f"st_{name}")
                for a in range(4):
                    nc.vector.bn_stats(st[:, a, :], xt[:, 512 * a:512 * (a + 1)])
                mv = p_ln.tile([P, 2], F32, tag="lnmv", name=f"mv_{name}")
                nc.vector.bn_aggr(mv[:], st[:])
                sd = p_ln.tile([P, 1], F32, tag="lnsd", name=f"sd_{name}")
                nc.scalar.activation(sd[:], mv[:, 1:2],
                                     mybir.ActivationFunctionType.Sqrt, bias=epsb[:])
                rstd = p_ln.tile([P, 1], F32, tag="lnrstd", name=f"rstd_{name}")
                nc.vector.reciprocal(rstd[:], sd[:])
                nmu = p_ln.tile([P, 1], F32, tag="lnnmu", name=f"nmu_{name}")
                nc.vector.tensor_tensor(nmu[:], mv[:, 0:1], rstd[:], mybir.AluOpType.mult)
                nc.vector.tensor_scalar_mul(nmu[:], nmu[:], -1.0)
                ht = p_h.tile([P, H], BF16, tag="ht", name=f"ht_{name}")
                nc.scalar.activation(ht[:], xt[:],
                                     mybir.ActivationFunctionType.Identity,
                                     bias=nmu[:], scale=rstd[:])
                return ht

            def layer_norm_halves(xh, name):
                """xh: two [P, H/2] f32 SBUF halves -> bf16 [P, H] normalized."""
                st = p_ln.tile([P, 4, 6], F32, tag="lnst", name=f"st_{name}")
                for hh in range(2):
                    for a in range(2):
                        nc.vector.bn_stats(st[:, 2 * hh + a, :],
                                           xh[hh][:, 512 * a:512 * (a + 1)])
                mv = p_ln.tile([P, 2], F32, tag="lnmv", name=f"mv_{name}")
                nc.vector.bn_aggr(mv[:], st[:])
                sd = p_ln.tile([P, 1], F32, tag="lnsd", name=f"sd_{name}")
                nc.scalar.activation(sd[:], mv[:, 1:2],
                                     mybir.ActivationFunctionType.Sqrt, bias=epsb[:])
                rstd = p_ln.tile([P, 1], F32, tag="lnrstd", name=f"rstd_{name}")
                nc.vector.reciprocal(rstd[:], sd[:])
                nmu = p_ln.tile([P, 1], F32, tag="lnnmu", name=f"nmu_{name}")
                nc.vector.tensor_tensor(nmu[:], mv[:, 0:1], rstd[:],
                                        mybir.AluOpType.mult)
                nc.vector.tensor_scalar_mul(nmu[:], nmu[:], -1.0)
                ht = p_h.tile([P, H], BF16, tag="ht", name=f"ht_{name}")
                for hh in range(2):
                    nc.scalar.activation(ht[:, (H // 2) * hh:(H // 2) * (hh + 1)],
                                         xh[hh][:],
                                         mybir.ActivationFunctionType.Identity,
                                         bias=nmu[:], scale=rstd[:])
                return ht

            # ================= Phase 1+2: LN1, transpose, QKV =================
            ksb = [None, None]   # per-batch K tiles [P, 2, TPB] bf16
            vsb = [None, None]   # per-batch V tiles [P, 16, DV] bf16 (token-major)
            hts = [None] * (NTOK // 512)

            for nt in range(NTOK // 512):   # 512-token groups
                b = nt // 4
                if nt % 4 == 0:
                    ksb[b] = p_k.tile([P, HL, TPB], BF16, tag="ksb",
                                      name=f"ksb_{b}")
                    vsb[b] = p_k.tile([P, QT_PER_B, DV], BF16, tag="vsb",
                                      name=f"vsb_{b}")
                hT = p_hT.tile([P, KS, 512], BF16, tag="hT", name=f"hT_{nt}")
                hts[nt] = hT
                for tt in range(4):         # 128-token LN tiles
                    t = 4 * nt + tt
                    xh = []
                    for hh in range(2):
                        xth = p_x.tile([P, H // 2], F32, tag="xt",
                                       name=f"xt_{t}_{hh}")
                        nc.sync.dma_start(
                            xth[:], x_d[P * t:P * (t + 1),
                                        (H // 2) * hh:(H // 2) * (hh + 1)])
                        xh.append(xth)
                    ht = layer_norm_halves(xh, f"ln1_{t}")
                    for fg in range(KS // 4):
                        ptp = psA.tile([P, 512], BF16, tag="psA", name=f"trp_{t}_{fg}")
                        for f4 in range(4):
                            f = 4 * fg + f4
                            nc.tensor.transpose(ptp[:, P * f4:P * (f4 + 1)],
                                                ht[:, P * f:P * (f + 1)], ident[:])
                        nc.any.tensor_copy(
                            out=hT[:, 4 * fg:4 * (fg + 1), P * tt:P * (tt + 1)],
                            in_=ptp[:].rearrange("p (a b) -> p a b", b=P))

                # QK projection: out rows m (0,1 -> Q head0/1 ; 2,3 -> K head0/1)
                col0 = 512 * (nt % 4)
                for m in range(4):
                    ps = psA.tile([P, 512], F32, tag="psA", name=f"qk_{nt}_{m}")
                    for ks in range(KS):
                        nc.tensor.matmul(ps[:], lhsT=wqk_sb[:, ks, P * m:P * (m + 1)],
                                         rhs=hT[:, ks, :],
                                         start=(ks == 0), stop=(ks == KS - 1))
                    if m < 2:
                        qsl = p_qv.tile([P, 512], BF16, tag="qsl", name=f"qsl_{nt}_{m}")
                        nc.scalar.activation(qsl[:], ps[:],
                                             mybir.ActivationFunctionType.Identity,
                                             bias=bqk_sb[:, m:m + 1])
                        nc.sync.dma_start(qT_dram[P * m:P * (m + 1),
                                                  512 * nt:512 * (nt + 1)], qsl[:])
                    else:
                        nc.scalar.activation(ksb[b][:, m - 2, col0:col0 + 512], ps[:],
                                             mybir.ActivationFunctionType.Identity,
                                             bias=bqk_sb[:, m:m + 1])
                # V projection (token-major)
                for m in range(4):
                    ps = psA.tile([P, 512], F32, tag="psA", name=f"v_{nt}_{m}")
                    for ks in range(KS):
                        nc.tensor.matmul(ps[:, :DV], lhsT=hT[:, ks, P * m:P * (m + 1)],
                                         rhs=wv_sb[:, ks, :],
                                         start=(ks == 0), stop=(ks == KS - 1))
                    tm = (4 * nt + m) % QT_PER_B
                    nc.vector.tensor_tensor(vsb[b][:, tm, :], ps[:, :DV], bvbc_sb[:],
                                            mybir.AluOpType.add)

            # ================= Phase 3: attention + O-projection ==============
            rg = [list(range(NCORES))]
            for b in range(B):
                for qt in range(QT_PER_B):
                    mt = QT_PER_B * b + qt          # token m-tile index
                    klen = P * (qt + 1)
                    tok0 = TPB * b
                    aot = p_at.tile([P, HL, P], BF16, tag="aot", name=f"aot_{mt}")
                    qblk = p_blk.tile([P, HL, P], BF16, tag="qblk", name=f"qb_{mt}")
                    nc.sync.dma_start(
                        qblk[:],
                        qT_dram[:, tok0 + P * qt:tok0 + P * (qt + 1)]
                        .rearrange("(a p) t -> p a t", p=P))
                    for lh in range(HL):
                        # softmax without max-subtraction: post-LN logits here are
                        # O(5), far from f32 exp overflow; masked lanes are -1e30.
                        nchs = (qt + 4) // 4
                        pex = p_at.tile([P, TPB], BF16, tag="pex",
                                        name=f"pex_{mt}_{lh}")
                        sexp = p_ln.tile([P, 4], F32, tag="sexp",
                                         name=f"se_{mt}_{lh}")
                        for j in range(nchs):
                            n0 = 512 * j
                            n1 = min(n0 + 512, klen)
                            ps = psA.tile([P, 512], F32, tag="psA",
                                          name=f"s_{mt}_{lh}_{j}")
                            nc.tensor.matmul(ps[:, :n1 - n0], lhsT=qblk[:, lh, :],
                                             rhs=ksb[b][:, lh, n0:n1],
                                             start=True, stop=True)
                            if j == nchs - 1:
                                d0 = klen - P - n0
                                nc.vector.tensor_tensor(ps[:, d0:d0 + P],
                                                        ps[:, d0:d0 + P],
                                                        cmask[:], mybir.AluOpType.add)
                            nc.scalar.activation(pex[:, n0:n1], ps[:, :n1 - n0],
                                                 mybir.ActivationFunctionType.Exp,
                                                 accum_out=sexp[:, j:j + 1])
                        rinv = p_ln.tile([P, 1], F32, tag="rinv",
                                         name=f"ri_{mt}_{lh}")
                        if nchs > 1:
                            ssum = p_ln.tile([P, 1], F32, tag="ssum",
                                             name=f"ss_{mt}_{lh}")
                            nc.vector.tensor_reduce(ssum[:], sexp[:, :nchs],
                                                    axis=mybir.AxisListType.X,
                                                    op=mybir.AluOpType.add)
                            nc.vector.reciprocal(rinv[:], ssum[:])
                        else:
                            nc.vector.reciprocal(rinv[:], sexp[:, 0:1])
                        nc.vector.tensor_scalar_mul(pex[:, :klen], pex[:, :klen],
                                                    rinv[:])
                        ps_o = psA.tile([P, P], F32, tag="psA", name=f"o_{mt}_{lh}")
                        for kg in range((qt + 4) // 4):   # groups of 4 k-blocks
                            g = min(4, qt + 1 - 4 * kg)
                            ptp = psA.tile([P, 512], BF16, tag="psA",
                                           name=f"pt_{mt}_{lh}_{kg}")
                            for k4 in range(g):
                                kb = 4 * kg + k4
                                nc.tensor.transpose(ptp[:, P * k4:P * (k4 + 1)],
                                                    pex[:, P * kb:P * (kb + 1)],
                                                    ident[:])
                            pts = p_blk.tile([P, 512], BF16, tag="pts",
                                             name=f"pts_{mt}_{lh}_{kg}")
                            nc.vector.tensor_copy(out=pts[:, :P * g], in_=ptp[:, :P * g])
                            for k4 in range(g):
                                kb = 4 * kg + k4
                                nc.tensor.matmul(ps_o[:],
                                                 lhsT=vsb[b][:, kb, P * lh:P * (lh + 1)],
                                                 rhs=pts[:, P * k4:P * (k4 + 1)],
                                                 start=(kb == 0), stop=(kb == qt))
                        nc.vector.tensor_copy(out=aot[:, lh, :], in_=ps_o[:])
                    # O projection partial for this 128-token tile
                    r0 = P * mt
                    for nh in range(2):
                        po_t = p_qv.tile([P, H // 2], BF16, tag="po_t",
                                         name=f"po_{mt}_{nh}")
                        for n2 in range(2):
                            nk = 2 * nh + n2
                            ps = psA.tile([P, 512], F32, tag="psA",
                                          name=f"po_{mt}_{nk}")
                            for ks in range(HL):
                                nc.tensor.matmul(
                                    ps[:], lhsT=aot[:, ks, :],
                                    rhs=wo_sb[:, ks, 512 * nk:512 * (nk + 1)],
                                    start=(ks == 0), stop=(ks == HL - 1))
                            nc.any.tensor_copy(out=po_t[:, 512 * n2:512 * (n2 + 1)],
                                               in_=ps[:])
                        nc.sync.dma_start(
                            po_dram[r0:r0 + P, 1024 * nh:1024 * (nh + 1)], po_t[:])
                    # issue reduce-scatter when a 1024-token chunk is complete
                    if mt % (MT // NCHUNK) == MT // NCHUNK - 1:
                        j = mt // (MT // NCHUNK)
                        nc.gpsimd.collective_compute(
                            "ReduceScatter", mybir.AluOpType.add, replica_groups=rg,
                            ins=[po_dram[1024 * j:1024 * (j + 1), :]],
                            outs=[rs_dram[TOKC * j:TOKC * (j + 1), :]])

            # ================= Phase 4: residual + LN2 + MLP ==================
            h2T = [None, None]   # [P, KS, 256] per chunk pair
            b2h = []
            for hh in range(2):
                b2t = p_x.tile([P, H // 2], F32, tag="xt", name=f"b2t_{hh}")
                nc.sync.dma_start(b2t[:], b2bc_d[:, (H // 2) * hh:(H // 2) * (hh + 1)])
                b2h.append(b2t)
            for j in range(NCHUNK):
                x2 = p_mlp.tile([P, H], F32, tag="x2", name=f"x2_{j}")
                nc.sync.dma_start(x2[:], xres_d[P * j:P * (j + 1), :])
                rsj = p_rs.tile([P, H], BF16, tag="rsj", name=f"rsj_{j}")
                nc.sync.dma_start(rsj[:], rs_dram[P * j:P * (j + 1), :])
                nc.vector.tensor_tensor(x2[:], x2[:], rsj[:], mybir.AluOpType.add)
                h2 = layer_norm_tile(x2, f"ln2_{j}")
                # after LN2 consumed x2, fold b2 in-place and spill for MLP2 residual
                for hh in range(2):
                    cs = slice((H // 2) * hh, (H // 2) * (hh + 1))
                    nc.vector.tensor_tensor(x2[:, cs], x2[:, cs], b2h[hh][:],
                                            mybir.AluOpType.add)
                nc.sync.dma_start(x2_dram[P * j:P * (j + 1), :], x2[:])
                jg, jj = j // 2, j % 2
                if jj == 0:
                    h2T[jg] = p_mlp.tile([P, KS, 256], BF16, tag="h2T",
                                         name=f"h2T_{jg}")
                for fg in range(KS // 4):
                    ptp = psA.tile([P, 512], BF16, tag="psA", name=f"h2t_{j}_{fg}")
                    for f4 in range(4):
                        f = 4 * fg + f4
                        nc.tensor.transpose(ptp[:, P * f4:P * (f4 + 1)],
                                            h2[:, P * f:P * (f + 1)], ident[:])
                    nc.any.tensor_copy(
                        out=h2T[jg][:, 4 * fg:4 * (fg + 1), P * jj:P * (jj + 1)],
                        in_=ptp[:].rearrange("p (a b) -> p a b", b=P))

            # MLP1: U_T[mid, tok] = silu(W1_eff.T @ h2T + b1), spilled to DRAM
            silu_fn = (mybir.ActivationFunctionType.Sigmoid if sim
                       else mybir.ActivationFunctionType.Silu)
            for jg in range(2):
                for mm in range(MMT):
                    w1t = p_st.tile([P, KS, P], BF16, tag="w1t",
                                    name=f"w1t_{jg}_{mm}")
                    nc.gpsimd.dma_start(out=w1t[:], in_=w1_d[mm, :, :, :])
                    usl = p_st.tile([P, 256], BF16, tag="usl", name=f"usl_{jg}_{mm}")
                    ps = psA.tile([P, 512], F32, tag="psA", name=f"u_{jg}_{mm}")
                    for ks in range(KS):
                        nc.tensor.matmul(ps[:, :256],
                                         lhsT=w1t[:, ks, :],
                                         rhs=h2T[jg][:, ks, :],
                                         start=(ks == 0), stop=(ks == KS - 1))
                    nc.scalar.activation(usl[:], ps[:, :256],
                                         silu_fn, bias=b1_sb[:, mm:mm + 1])
                    nc.sync.dma_start(
                        ut_dram[P * mm:P * (mm + 1), 256 * jg:256 * (jg + 1)],
                        usl[:])

            # MLP2: out[tok, H] = U_T.T @ W2 + b2 + x2
            for jg in range(2):
                for ng in range(2):
                    pss = [[None] * 2 for _ in range(2)]
                    for jj in range(2):
                        for nn in range(2):
                            pss[jj][nn] = psA.tile([P, 512], F32, tag="psA",
                                                   name=f"y_{jg}_{ng}_{jj}_{nn}")
                    for kg in range(MMT // 2):
                        utk = p_st.tile([P, 2, 256], BF16, tag="utk",
                                        name=f"utk_{jg}_{ng}_{kg}")
                        nc.sync.dma_start(
                            utk[:], ut_dram[256 * kg:256 * (kg + 1),
                                            256 * jg:256 * (jg + 1)]
                            .rearrange("(a p) t -> p a t", p=P))
                        w2t = p_st.tile([P, 2, 1024], BF16, tag="w2t",
                                        name=f"w2t_{jg}_{ng}_{kg}")
                        nc.gpsimd.dma_start(
                            out=w2t[:], in_=w2_d[256 * kg:256 * (kg + 1),
                                                 1024 * ng:1024 * (ng + 1)]
                            .rearrange("(a p) n -> p a n", p=P))
                        for k4 in range(2):
                            ks = 2 * kg + k4
                            for jj in range(2):
                                for nn in range(2):
                                    nc.tensor.matmul(
                                        pss[jj][nn][:],
                                        lhsT=utk[:, k4, P * jj:P * (jj + 1)],
                                        rhs=w2t[:, k4, 512 * nn:512 * (nn + 1)],
                                        start=(ks == 0), stop=(ks == MMT - 1))
                    for jj in range(2):
                        j = 2 * jg + jj
                        for nn in range(2):
                            c0 = 1024 * ng + 512 * nn
                            x2sl = p_st.tile([P, 512], F32, tag="x2sl",
                                             name=f"x2sl_{j}_{ng}_{nn}")
                            nc.sync.dma_start(x2sl[:],
                                              x2_dram[P * j:P * (j + 1), c0:c0 + 512])
                            ot = p_st.tile([P, 512], F32, tag="ot",
                                           name=f"ot_{j}_{ng}_{nn}")
                            nc.vector.tensor_tensor(ot[:], pss[jj][nn][:], x2sl[:],
                                                    mybir.AluOpType.add)
                            nc.sync.dma_start(out_d[P * j:P * (j + 1), c0:c0 + 512],
                                              ot[:])
    nc.compile()
    return nc


def _bf16(a):
    return np.asarray(a, dtype=np.float32).astype(ml_dtypes.bfloat16)


def make_in_maps(x, Wq, Wk, Wv, Wo, g1, bn1, g2, bn2, W1, b1, W2, b2):
    x = np.asarray(x, np.float32)
    x_flat = np.ascontiguousarray(x.reshape(NTOK, H))
    s = np.float32(1.0 / np.sqrt(P))

    wq_eff = (g1[:, None] * np.asarray(Wq, np.float32)) * s
    wk_eff = g1[:, None] * np.asarray(Wk, np.float32)
    wv_eff = g1[:, None] * np.asarray(Wv, np.float32)
    bq = (bn1 @ np.asarray(Wq, np.float32)) * s
    bk = bn1 @ np.asarray(Wk, np.float32)
    bv = bn1 @ np.asarray(Wv, np.float32)
    w1_eff = g2[:, None] * np.asarray(W1, np.float32)
    b1_eff = np.asarray(b1, np.float32) + bn2 @ np.asarray(W1, np.float32)

    # shared tensors
    w1_t = np.ascontiguousarray(
        _bf16(w1_eff).reshape(KS, P, MMT, P).transpose(2, 1, 0, 3))  # [mm, p, ks, mw]
    w2_t = np.ascontiguousarray(_bf16(W2))
    b1m = np.ascontiguousarray(b1_eff.reshape(MMT, P).T.astype(np.float32))
    b2bc = np.ascontiguousarray(
        np.broadcast_to(np.asarray(b2, np.float32), (P, H)))
    ii, jj_ = np.meshgrid(np.arange(P), np.arange(P), indexing="ij")
    cmask = np.where(jj_ <= ii, 0.0, NEG).astype(np.float32)

    in_maps = []
    for c in range(NCORES):
        cs = slice(DV * c, DV * (c + 1))
        wqk = np.concatenate([wq_eff[:, cs], wk_eff[:, cs]], axis=1)  # [H, 512]
        wqk_t = np.ascontiguousarray(
            _bf16(wqk).reshape(KS, P, DQK).transpose(1, 0, 2))
        bqk = np.concatenate([bq[cs], bk[cs]]).astype(np.float32)
        bqk_m = np.ascontiguousarray(bqk.reshape(DQK // P, P).T)
        wv_t = np.ascontiguousarray(
            _bf16(wv_eff[:, cs]).reshape(KS, P, DV).transpose(1, 0, 2))
        bvbc = np.ascontiguousarray(
            np.broadcast_to(bv[cs].astype(np.float32), (P, DV)))
        wo_t = np.ascontiguousarray(
            _bf16(np.asarray(Wo, np.float32)[cs, :]).reshape(DV // P, P, H)
            .transpose(1, 0, 2))
        xres = np.concatenate(
            [x_flat[1024 * j + P * c:1024 * j + P * (c + 1)] for j in range(NCHUNK)],
            axis=0)
        in_maps.append({
            "x": x_flat, "xres": np.ascontiguousarray(xres),
            "wqk": wqk_t, "bqk": bqk_m, "wv": wv_t, "bvbc": bvbc, "wo": wo_t,
            "w1": w1_t, "b1": b1m, "w2": w2_t, "b2bc": b2bc, "cmask": cmask,
        })
    return in_maps


_NC_CACHE = {}


def kernel(**inputs):
    if "nc" not in _NC_CACHE:
        _NC_CACHE["nc"] = build()
    nc = _NC_CACHE["nc"]
    in_maps = make_in_maps(
        inputs["x"], inputs["Wq"], inputs["Wk"], inputs["Wv"], inputs["Wo"],
        np.asarray(inputs["g1"], np.float32), np.asarray(inputs["bn1"], np.float32),
        np.asarray(inputs["g2"], np.float32), np.asarray(inputs["bn2"], np.float32),
        inputs["W1"], inputs["b1"], inputs["W2"], inputs["b2"])
    res = run_bass_kernel_spmd(nc, in_maps, list(range(NCORES)))
    out = np.empty((NTOK, H), np.float32)
    for c in range(NCORES):
        oc = res.results[c]["out"]
        for j in range(NCHUNK):
            out[1024 * j + P * c:1024 * j + P * (c + 1)] = oc[P * j:P * (j + 1)]
    return out.reshape(B, T, H)
